# revision 4
# baseline (speedup 1.0000x reference)
"""Trainium2 Bass kernel for residual vector quantization (RVQ) with input/output
projections, distributed data-parallel over batch across 8 NeuronCores.

Reference computation (per full input):
    h = tanh(x @ Wi + bi) * 8                                [B, T, DC]
    residual = h; quant_out = 0
    for q in range(NQ):
        d = ||r||^2 - 2 r.cb[q]^T + ||cb[q]||^2              [B, T, K]
        idx = argmin_K d ; quant = cb[q][idx]
        loss_q = mean((quant - r)^2)  (== mean(new_r^2))
        quant_out += quant ; r -= quant
    out = quant_out @ Wo + bo
Returns (out, codes [NQ,B,T] int32, commit_loss scalar, sub [NQ,B,DC,T]).

Sharding: batch B=16 split 2-per-core across 8 cores; weights/codebooks
replicated. Final tiny reductions (loss partials) summed on host.
"""

import numpy as np

import concourse.bass as bass
import concourse.bacc as bacc
import concourse.tile as tile
from concourse import mybir
from concourse.bass_utils import run_bass_kernel_spmd
from concourse.masks import make_identity

# Problem constants (hardcoded per harness contract)
B, T, DIN, DC, NQ, K = 16, 2000, 1024, 512, 8, 1024
CODEC_RANGE = 8.0
N_CORES = 8
BPC = B // N_CORES            # batch samples per core
F = BPC * T                   # frames per core = 4000
TT = 125                      # frame tile (2000 = 16*125; no batch-boundary straddle)
NMT = F // TT                 # 32 frame tiles per core
MT_PER_B = T // TT            # 16 tiles per batch sample
FCH = 500                     # phase-A frame chunk
NFCH = F // FCH               # 8 chunks
KB = DC // 128                # 4 contraction tiles of 128 over DC
KT = DIN // 128               # 8 contraction tiles of 128 over DIN

FP32 = mybir.dt.float32
U32 = mybir.dt.uint32
I32 = mybir.dt.int32
NEG_INF = -3.0e38


def build_bass():
    nc = bacc.Bacc("TRN2", target_bir_lowering=False, debug=False)

    x_in = nc.dram_tensor("x", [F, DIN], FP32, kind="ExternalInput")
    wi_in = nc.dram_tensor("Wi", [DIN, DC], FP32, kind="ExternalInput")
    bi_in = nc.dram_tensor("bi", [DC], FP32, kind="ExternalInput")
    wo_in = nc.dram_tensor("Wo", [DC, DIN], FP32, kind="ExternalInput")
    bo_in = nc.dram_tensor("bo", [DIN], FP32, kind="ExternalInput")
    # per-quantizer codebooks: row-major (gather source) and transposed (matmul rhs)
    cb_in = [
        nc.dram_tensor(f"cb{q}", [K, DC], FP32, kind="ExternalInput") for q in range(NQ)
    ]
    cbt_in = [
        nc.dram_tensor(f"cbT{q}", [DC, K], FP32, kind="ExternalInput")
        for q in range(NQ)
    ]
    # 0.5 * ||cb||^2 per row, host-precomputed
    hsq_in = nc.dram_tensor("hsq", [NQ, K], FP32, kind="ExternalInput")

    out_o = nc.dram_tensor("out", [F, DIN], FP32, kind="ExternalOutput")
    codes_o = nc.dram_tensor("codes", [NQ, F], I32, kind="ExternalOutput")
    sub_o = nc.dram_tensor("sub", [NQ, BPC, DC, T], FP32, kind="ExternalOutput")
    lossp_o = nc.dram_tensor("lossp", [128, NQ * NMT], FP32, kind="ExternalOutput")

    with tile.TileContext(nc) as tc:
        with (
            tc.tile_pool(name="persist", bufs=1) as persist,
            tc.tile_pool(name="work", bufs=2) as work,
        ):
            ident = persist.tile([128, 128], FP32)
            make_identity(nc, ident[:])

            # residual rT and accumulated quant qT, transposed layout:
            # [partition = dc%128, kb = dc//128, frame]
            rT = persist.tile([128, KB, F], FP32)
            qT = persist.tile([128, KB, F], FP32)
            lstage = persist.tile([128, NQ * NMT], FP32)
            nc.vector.memset(lstage[:], 0.0)

            bi_sb = persist.tile([128, KB], FP32)
            nc.sync.dma_start(
                out=bi_sb[:], in_=bi_in[:].rearrange("(kb p) -> p kb", p=128)
            )

            # ---------------- Phase A: h = tanh(x @ Wi + bi) * 8 -> rT -------
            with (
                tc.tile_pool(name="pa_sb", bufs=2) as pa_sb,
                tc.tile_pool(name="pa_wi", bufs=1) as pa_wi,
                tc.tile_pool(name="pa_ps", bufs=2, space="PSUM") as pa_ps,
                tc.tile_pool(name="pa_psh", bufs=3, space="PSUM") as pa_psh,
            ):
                wi_sb = pa_wi.tile([128, KT, DC], FP32)
                nc.sync.dma_start(
                    out=wi_sb[:], in_=wi_in[:].rearrange("(kt p) d -> p kt d", p=128)
                )

                for fc in range(NFCH):
                    xt_sb = pa_sb.tile([128, KT, FCH], FP32, tag="xt")
                    for s in range(FCH // TT):
                        f0 = fc * FCH + s * TT
                        xrow = pa_sb.tile([128, DIN], FP32, tag="xrow")
                        nc.sync.dma_start(
                            out=xrow[:TT, :], in_=x_in[f0 : f0 + TT, :]
                        )
                        # middle stride 128 keeps each transpose inside one psum bank
                        xt_ps = pa_ps.tile([128, KT, 128], FP32)
                        for kt in range(KT):
                            nc.tensor.transpose(
                                out=xt_ps[:, kt, :TT],
                                in_=xrow[:TT, kt * 128 : (kt + 1) * 128],
                                identity=ident[:TT, :TT],
                            )
                        nc.vector.tensor_copy(
                            out=xt_sb[:, :, s * TT : (s + 1) * TT], in_=xt_ps[:, :, :TT]
                        )
                    for dct in range(KB):
                        h_ps = pa_psh.tile([128, FCH], FP32)
                        for kt in range(KT):
                            nc.tensor.matmul(
                                h_ps[:],
                                lhsT=wi_sb[:, kt, dct * 128 : (dct + 1) * 128],
                                rhs=xt_sb[:, kt, :],
                                start=(kt == 0),
                                stop=(kt == KT - 1),
                            )
                        dst = rT[:, dct, fc * FCH : (fc + 1) * FCH]
                        nc.scalar.activation(
                            out=dst,
                            in_=h_ps[:],
                            func=mybir.ActivationFunctionType.Tanh,
                            bias=bi_sb[:, dct : dct + 1],
                        )
                        nc.scalar.mul(out=dst, in_=dst, mul=CODEC_RANGE)

            # ---------------- Phase B: NQ rounds of VQ ----------------------
            with (
                tc.tile_pool(name="pb_cb", bufs=2) as pb_cb,
                tc.tile_pool(name="pb_sb", bufs=3) as pb_sb,
                tc.tile_pool(name="pb_ps", bufs=2, space="PSUM") as pb_ps,
                tc.tile_pool(name="pb_psq", bufs=2, space="PSUM") as pb_psq,
            ):
                for q in range(NQ):
                    cbt_sb = pb_cb.tile([128, KB, K], FP32, tag="cbt")
                    nc.sync.dma_start(
                        out=cbt_sb[:],
                        in_=cbt_in[q][:].rearrange("(kb p) k -> p kb k", p=128),
                    )
                    hsq_b = pb_cb.tile([128, K], FP32, tag="hsq")
                    nc.sync.dma_start(
                        out=hsq_b[:],
                        in_=bass.AP(
                            tensor=hsq_in,
                            offset=q * K,
                            ap=[[0, 128], [1, K]],
                        ),
                    )

                    for mt in range(NMT):
                        b, tb = mt // MT_PER_B, mt % MT_PER_B
                        sl = slice(mt * TT, (mt + 1) * TT)

                        # scores t = r . c  ->  psum [TT, K]
                        ps_d = pb_ps.tile([128, K], FP32, tag="d")
                        for kb in range(KB):
                            for nb in range(2):
                                nc.tensor.matmul(
                                    ps_d[:TT, nb * 512 : (nb + 1) * 512],
                                    lhsT=rT[:, kb, sl],
                                    rhs=cbt_sb[:, kb, nb * 512 : (nb + 1) * 512],
                                    start=(kb == 0),
                                    stop=(kb == KB - 1),
                                )
                        # u = t - 0.5*||c||^2 (in place in psum), then argmax
                        nc.vector.tensor_tensor(
                            out=ps_d[:TT, :],
                            in0=ps_d[:TT, :],
                            in1=hsq_b[:TT, :],
                            op=mybir.AluOpType.subtract,
                        )
                        mx8 = pb_sb.tile([128, 8], FP32, tag="mx8")
                        idx8 = pb_sb.tile([128, 8], U32, tag="idx8")
                        nc.vector.max(mx8[:TT, :], ps_d[:TT, :])
                        nc.vector.max_index(idx8[:TT, :], mx8[:TT, :], ps_d[:TT, :])
                        nc.sync.dma_start(
                            out=codes_o[q, sl],
                            in_=idx8[:TT, 0:1].bitcast(I32),
                        )
                        # gather quant rows from codebook (DRAM), [TT, DC]
                        qrow = pb_sb.tile([128, DC], FP32, tag="qrow")
                        nc.gpsimd.indirect_dma_start(
                            out=qrow[:TT, :],
                            out_offset=None,
                            in_=cb_in[q][:],
                            in_offset=bass.IndirectOffsetOnAxis(
                                ap=idx8[:TT, 0:1], axis=0
                            ),
                        )
                        # transpose quant -> [dc, frame] layout
                        ps_q = pb_psq.tile([128, KB, TT], FP32)
                        for i in range(KB):
                            nc.tensor.transpose(
                                out=ps_q[:, i, :],
                                in_=qrow[:TT, i * 128 : (i + 1) * 128],
                                identity=ident[:TT, :TT],
                            )
                        qtc = pb_sb.tile([128, KB, TT], FP32, tag="qtc")
                        nc.scalar.copy(out=qtc[:], in_=ps_q[:])
                        # sub[q, b, :, tb*TT:+TT] = quant^T
                        nc.sync.dma_start(
                            out=sub_o[q, b].rearrange("(i p) t -> p i t", p=128)[
                                :, :, tb * TT : (tb + 1) * TT
                            ],
                            in_=qtc[:],
                        )
                        # residual update and quant accumulation (gpsimd, SBUF)
                        nc.gpsimd.tensor_tensor(
                            out=rT[:, :, sl],
                            in0=rT[:, :, sl],
                            in1=qtc[:],
                            op=mybir.AluOpType.subtract,
                        )
                        if q == 0:
                            nc.gpsimd.tensor_copy(out=qT[:, :, sl], in_=qtc[:])
                        else:
                            nc.gpsimd.tensor_tensor(
                                out=qT[:, :, sl],
                                in0=qT[:, :, sl],
                                in1=qtc[:],
                                op=mybir.AluOpType.add,
                            )
                        # loss partial: sum(new_residual^2) over this tile
                        sq = pb_sb.tile([128, KB, TT], FP32, tag="sq")
                        nc.scalar.activation(
                            out=sq[:],
                            in_=rT[:, :, sl],
                            func=mybir.ActivationFunctionType.Square,
                            accum_out=lstage[:, q * NMT + mt : q * NMT + mt + 1],
                        )

            nc.sync.dma_start(out=lossp_o[:], in_=lstage[:])

            # ---------------- Phase C: out = qT^T @ Wo + bo ------------------
            with (
                tc.tile_pool(name="pc_w", bufs=1) as pc_w,
                tc.tile_pool(name="pc_sb", bufs=3) as pc_sb,
                tc.tile_pool(name="pc_ps", bufs=2, space="PSUM") as pc_ps,
            ):
                wo_sb = pc_w.tile([128, KB, DIN], FP32)
                nc.sync.dma_start(
                    out=wo_sb[:], in_=wo_in[:].rearrange("(kb p) n -> p kb n", p=128)
                )
                bo_b = pc_w.tile([128, DIN], FP32)
                nc.sync.dma_start(
                    out=bo_b[:],
                    in_=bass.AP(tensor=bo_in, offset=0, ap=[[0, 128], [1, DIN]]),
                )
                for mt in range(NMT):
                    sl = slice(mt * TT, (mt + 1) * TT)
                    ps_o = pc_ps.tile([128, DIN], FP32)
                    for kb in range(KB):
                        for nb in range(2):
                            nc.tensor.matmul(
                                ps_o[:TT, nb * 512 : (nb + 1) * 512],
                                lhsT=qT[:, kb, sl],
                                rhs=wo_sb[:, kb, nb * 512 : (nb + 1) * 512],
                                start=(kb == 0),
                                stop=(kb == KB - 1),
                            )
                    osb = pc_sb.tile([128, DIN], FP32)
                    nc.vector.tensor_tensor(
                        out=osb[:TT, :],
                        in0=ps_o[:TT, :],
                        in1=bo_b[:TT, :],
                        op=mybir.AluOpType.add,
                    )
                    nc.sync.dma_start(out=out_o[sl, :], in_=osb[:TT, :])

    nc.compile()
    return nc


_NC_CACHE = None


def _get_nc():
    global _NC_CACHE
    if _NC_CACHE is None:
        _NC_CACHE = build_bass()
    return _NC_CACHE


def kernel(x, Wi, bi, Wo, bo, codebooks):
    x = np.ascontiguousarray(np.asarray(x, dtype=np.float32))
    Wi = np.ascontiguousarray(np.asarray(Wi, dtype=np.float32))
    bi = np.ascontiguousarray(np.asarray(bi, dtype=np.float32))
    Wo = np.ascontiguousarray(np.asarray(Wo, dtype=np.float32))
    bo = np.ascontiguousarray(np.asarray(bo, dtype=np.float32))
    codebooks = np.ascontiguousarray(np.asarray(codebooks, dtype=np.float32))

    hsq = 0.5 * np.sum(
        codebooks.astype(np.float64) * codebooks.astype(np.float64), axis=-1
    ).astype(np.float32)  # [NQ, K]

    nc = _get_nc()
    shared = {
        "Wi": Wi,
        "bi": bi,
        "Wo": Wo,
        "bo": bo,
        "hsq": hsq,
    }
    for q in range(NQ):
        shared[f"cb{q}"] = np.ascontiguousarray(codebooks[q])
        shared[f"cbT{q}"] = np.ascontiguousarray(codebooks[q].T)

    in_maps = []
    for c in range(N_CORES):
        m = dict(shared)
        m["x"] = np.ascontiguousarray(
            x[c * BPC : (c + 1) * BPC].reshape(F, DIN)
        )
        in_maps.append(m)

    res = run_bass_kernel_spmd(nc, in_maps, core_ids=list(range(N_CORES)))
    results = res.results

    out = np.empty((B, T, DIN), np.float32)
    codes = np.empty((NQ, B, T), np.int32)
    sub = np.empty((NQ, B, DC, T), np.float32)
    loss_sum = 0.0
    for c in range(N_CORES):
        r = results[c]
        out[c * BPC : (c + 1) * BPC] = r["out"].reshape(BPC, T, DIN)
        codes[:, c * BPC : (c + 1) * BPC, :] = r["codes"].reshape(NQ, BPC, T)
        sub[:, c * BPC : (c + 1) * BPC] = r["sub"]
        loss_sum += float(r["lossp"].astype(np.float64).sum())

    commit_loss = np.float32(loss_sum / (NQ * B * T * DC))
    return out, codes, commit_loss, sub


if __name__ == "__main__":
    # smoke test with random inputs
    rng = np.random.default_rng(0)
    inputs = {
        "x": rng.standard_normal((B, T, DIN), dtype=np.float32),
        "Wi": rng.standard_normal((DIN, DC), dtype=np.float32) / 32.0,
        "bi": np.zeros((DC,), np.float32),
        "Wo": rng.standard_normal((DC, DIN), dtype=np.float32) / 22.6,
        "bo": np.zeros((DIN,), np.float32),
        "codebooks": rng.standard_normal((NQ, K, DC), dtype=np.float32),
    }
    outs = kernel(**inputs)
    for o in outs:
        print(np.shape(o), np.asarray(o).dtype)


# revision 5
# speedup vs baseline: 134.3201x; 134.3201x over previous
"""Trainium2 Bass kernel for residual vector quantization (RVQ) with input/output
projections, distributed data-parallel over batch across 8 NeuronCores.

Reference computation (per full input):
    h = tanh(x @ Wi + bi) * 8                                [B, T, DC]
    residual = h; quant_out = 0
    for q in range(NQ):
        d = ||r||^2 - 2 r.cb[q]^T + ||cb[q]||^2              [B, T, K]
        idx = argmin_K d ; quant = cb[q][idx]
        loss_q = mean((quant - r)^2)  (== mean(new_r^2))
        quant_out += quant ; r -= quant
    out = quant_out @ Wo + bo
Returns (out, codes [NQ,B,T] int32, commit_loss scalar, sub [NQ,B,DC,T]).

Sharding: batch B=16 split 2-per-core across 8 cores; weights/codebooks
replicated. Final tiny reductions (loss partials) summed on host.
"""

import numpy as np

import concourse.bass as bass
import concourse.bacc as bacc
import concourse.tile as tile
from concourse import mybir
from concourse.bass_utils import run_bass_kernel_spmd
from concourse.masks import make_identity

# Problem constants (hardcoded per harness contract)
B, T, DIN, DC, NQ, K = 16, 2000, 1024, 512, 8, 1024
CODEC_RANGE = 8.0
N_CORES = 8
BPC = B // N_CORES            # batch samples per core
F = BPC * T                   # frames per core = 4000
TT = 125                      # frame tile (2000 = 16*125; no batch-boundary straddle)
NMT = F // TT                 # 32 frame tiles per core
MT_PER_B = T // TT            # 16 tiles per batch sample
FCH = 500                     # phase-A frame chunk
NFCH = F // FCH               # 8 chunks
KB = DC // 128                # 4 contraction tiles of 128 over DC
KT = DIN // 128               # 8 contraction tiles of 128 over DIN

FP32 = mybir.dt.float32
U32 = mybir.dt.uint32
I32 = mybir.dt.int32
NEG_INF = -3.0e38


def build_bass():
    nc = bacc.Bacc("TRN2", target_bir_lowering=False, debug=False)

    x_in = nc.dram_tensor("x", [F, DIN], FP32, kind="ExternalInput")
    wi_in = nc.dram_tensor("Wi", [DIN, DC], FP32, kind="ExternalInput")
    bi_in = nc.dram_tensor("bi", [DC], FP32, kind="ExternalInput")
    wo_in = nc.dram_tensor("Wo", [DC, DIN], FP32, kind="ExternalInput")
    bo_in = nc.dram_tensor("bo", [DIN], FP32, kind="ExternalInput")
    # per-quantizer codebooks: row-major (gather source) and transposed (matmul rhs)
    cb_in = [
        nc.dram_tensor(f"cb{q}", [K, DC], FP32, kind="ExternalInput") for q in range(NQ)
    ]
    cbt_in = [
        nc.dram_tensor(f"cbT{q}", [DC, K], FP32, kind="ExternalInput")
        for q in range(NQ)
    ]
    # 0.5 * ||cb||^2 per row, host-precomputed
    hsq_in = nc.dram_tensor("hsq", [NQ, K], FP32, kind="ExternalInput")

    out_o = nc.dram_tensor("out", [F, DIN], FP32, kind="ExternalOutput")
    codes_o = nc.dram_tensor("codes", [NQ, F], I32, kind="ExternalOutput")
    sub_o = nc.dram_tensor("sub", [NQ, BPC, DC, T], FP32, kind="ExternalOutput")
    lossp_o = nc.dram_tensor("lossp", [128, NQ * NMT], FP32, kind="ExternalOutput")

    with tile.TileContext(nc) as tc:
        with (
            tc.tile_pool(name="persist", bufs=1) as persist,
            tc.tile_pool(name="work", bufs=2) as work,
        ):
            ident = persist.tile([128, 128], FP32)
            make_identity(nc, ident[:])

            # residual rT and accumulated quant qT, transposed layout:
            # [partition = dc%128, kb = dc//128, frame]
            rT = persist.tile([128, KB, F], FP32)
            qT = persist.tile([128, KB, F], FP32)
            lstage = persist.tile([128, NQ * NMT], FP32)
            nc.vector.memset(lstage[:], 0.0)

            bi_sb = persist.tile([128, KB], FP32)
            nc.sync.dma_start(
                out=bi_sb[:], in_=bi_in[:].rearrange("(kb p) -> p kb", p=128)
            )

            # ---------------- Phase A: h = tanh(x @ Wi + bi) * 8 -> rT -------
            with (
                tc.tile_pool(name="pa_sb", bufs=2) as pa_sb,
                tc.tile_pool(name="pa_wi", bufs=1) as pa_wi,
                tc.tile_pool(name="pa_ps", bufs=2, space="PSUM") as pa_ps,
                tc.tile_pool(name="pa_psh", bufs=3, space="PSUM") as pa_psh,
            ):
                wi_sb = pa_wi.tile([128, KT, DC], FP32)
                nc.sync.dma_start(
                    out=wi_sb[:], in_=wi_in[:].rearrange("(kt p) d -> p kt d", p=128)
                )

                for fc in range(NFCH):
                    xt_sb = pa_sb.tile([128, KT, FCH], FP32, tag="xt")
                    for s in range(FCH // TT):
                        f0 = fc * FCH + s * TT
                        xrow = pa_sb.tile([128, DIN], FP32, tag="xrow")
                        nc.sync.dma_start(
                            out=xrow[:TT, :], in_=x_in[f0 : f0 + TT, :]
                        )
                        # middle stride 128 keeps each transpose inside one psum bank
                        xt_ps = pa_ps.tile([128, KT, 128], FP32)
                        for kt in range(KT):
                            nc.tensor.transpose(
                                out=xt_ps[:, kt, :TT],
                                in_=xrow[:TT, kt * 128 : (kt + 1) * 128],
                                identity=ident[:TT, :TT],
                            )
                        nc.vector.tensor_copy(
                            out=xt_sb[:, :, s * TT : (s + 1) * TT], in_=xt_ps[:, :, :TT]
                        )
                    for dct in range(KB):
                        h_ps = pa_psh.tile([128, FCH], FP32)
                        for kt in range(KT):
                            nc.tensor.matmul(
                                h_ps[:],
                                lhsT=wi_sb[:, kt, dct * 128 : (dct + 1) * 128],
                                rhs=xt_sb[:, kt, :],
                                start=(kt == 0),
                                stop=(kt == KT - 1),
                            )
                        dst = rT[:, dct, fc * FCH : (fc + 1) * FCH]
                        nc.scalar.activation(
                            out=dst,
                            in_=h_ps[:],
                            func=mybir.ActivationFunctionType.Tanh,
                            bias=bi_sb[:, dct : dct + 1],
                        )
                        nc.scalar.mul(out=dst, in_=dst, mul=CODEC_RANGE)

            # ---------------- Phase B: NQ rounds of VQ ----------------------
            with (
                tc.tile_pool(name="pb_cb", bufs=2) as pb_cb,
                tc.tile_pool(name="pb_sb", bufs=3) as pb_sb,
                tc.tile_pool(name="pb_ps", bufs=3, space="PSUM") as pb_ps,
                tc.tile_pool(name="pb_psq", bufs=2, space="PSUM") as pb_psq,
            ):
                for q in range(NQ):
                    cbt_sb = pb_cb.tile([128, KB, K], FP32, tag="cbt")
                    nc.sync.dma_start(
                        out=cbt_sb[:],
                        in_=cbt_in[q][:].rearrange("(kb p) k -> p kb k", p=128),
                    )
                    hsq_b = pb_cb.tile([128, K], FP32, tag="hsq")
                    nc.sync.dma_start(
                        out=hsq_b[:],
                        in_=bass.AP(
                            tensor=hsq_in,
                            offset=q * K,
                            ap=[[0, 128], [1, K]],
                        ),
                    )

                    for mt in range(NMT):
                        b, tb = mt // MT_PER_B, mt % MT_PER_B
                        sl = slice(mt * TT, (mt + 1) * TT)

                        # scores t = r . c  ->  psum [TT, K]
                        ps_d = pb_ps.tile([128, K], FP32, tag="d")
                        for kb in range(KB):
                            for nb in range(2):
                                nc.tensor.matmul(
                                    ps_d[:TT, nb * 512 : (nb + 1) * 512],
                                    lhsT=rT[:, kb, sl],
                                    rhs=cbt_sb[:, kb, nb * 512 : (nb + 1) * 512],
                                    start=(kb == 0),
                                    stop=(kb == KB - 1),
                                )
                        # u = t - 0.5*||c||^2 (in place in psum), then argmax
                        nc.vector.tensor_tensor(
                            out=ps_d[:TT, :],
                            in0=ps_d[:TT, :],
                            in1=hsq_b[:TT, :],
                            op=mybir.AluOpType.subtract,
                        )
                        mx8 = pb_sb.tile([128, 8], FP32, tag="mx8")
                        idx8 = pb_sb.tile([128, 8], U32, tag="idx8")
                        nc.vector.max(mx8[:TT, :], ps_d[:TT, :])
                        nc.vector.max_index(idx8[:TT, :], mx8[:TT, :], ps_d[:TT, :])
                        nc.sync.dma_start(
                            out=codes_o[q, sl],
                            in_=idx8[:TT, 0:1].bitcast(I32),
                        )
                        # gather quant rows from codebook (DRAM), [TT, DC]
                        qrow = pb_sb.tile([128, DC], FP32, tag="qrow")
                        nc.gpsimd.indirect_dma_start(
                            out=qrow[:TT, :],
                            out_offset=None,
                            in_=cb_in[q][:],
                            in_offset=bass.IndirectOffsetOnAxis(
                                ap=idx8[:TT, 0:1], axis=0
                            ),
                        )
                        # transpose quant -> [dc, frame] layout
                        ps_q = pb_psq.tile([128, KB, TT], FP32)
                        for i in range(KB):
                            nc.tensor.transpose(
                                out=ps_q[:, i, :],
                                in_=qrow[:TT, i * 128 : (i + 1) * 128],
                                identity=ident[:TT, :TT],
                            )
                        qtc = pb_sb.tile([128, KB, TT], FP32, tag="qtc")
                        nc.scalar.copy(out=qtc[:], in_=ps_q[:])
                        # sub[q, b, :, tb*TT:+TT] = quant^T
                        nc.sync.dma_start(
                            out=sub_o[q, b].rearrange("(i p) t -> p i t", p=128)[
                                :, :, tb * TT : (tb + 1) * TT
                            ],
                            in_=qtc[:],
                        )
                        # residual update and quant accumulation (gpsimd, SBUF)
                        nc.gpsimd.tensor_tensor(
                            out=rT[:, :, sl],
                            in0=rT[:, :, sl],
                            in1=qtc[:],
                            op=mybir.AluOpType.subtract,
                        )
                        if q == 0:
                            nc.gpsimd.tensor_copy(out=qT[:, :, sl], in_=qtc[:])
                        else:
                            nc.gpsimd.tensor_tensor(
                                out=qT[:, :, sl],
                                in0=qT[:, :, sl],
                                in1=qtc[:],
                                op=mybir.AluOpType.add,
                            )
                        # loss partial: sum(new_residual^2) over this tile
                        sq = pb_sb.tile([128, KB, TT], FP32, tag="sq")
                        nc.scalar.activation(
                            out=sq[:],
                            in_=rT[:, :, sl],
                            func=mybir.ActivationFunctionType.Square,
                            accum_out=lstage[:, q * NMT + mt : q * NMT + mt + 1],
                        )

            nc.sync.dma_start(out=lossp_o[:], in_=lstage[:])

            # ---------------- Phase C: out = qT^T @ Wo + bo ------------------
            with (
                tc.tile_pool(name="pc_w", bufs=1) as pc_w,
                tc.tile_pool(name="pc_sb", bufs=3) as pc_sb,
                tc.tile_pool(name="pc_ps", bufs=2, space="PSUM") as pc_ps,
            ):
                wo_sb = pc_w.tile([128, KB, DIN], FP32)
                nc.sync.dma_start(
                    out=wo_sb[:], in_=wo_in[:].rearrange("(kb p) n -> p kb n", p=128)
                )
                bo_b = pc_w.tile([128, DIN], FP32)
                nc.sync.dma_start(
                    out=bo_b[:],
                    in_=bass.AP(tensor=bo_in, offset=0, ap=[[0, 128], [1, DIN]]),
                )
                for mt in range(NMT):
                    sl = slice(mt * TT, (mt + 1) * TT)
                    ps_o = pc_ps.tile([128, DIN], FP32)
                    for kb in range(KB):
                        for nb in range(2):
                            nc.tensor.matmul(
                                ps_o[:TT, nb * 512 : (nb + 1) * 512],
                                lhsT=qT[:, kb, sl],
                                rhs=wo_sb[:, kb, nb * 512 : (nb + 1) * 512],
                                start=(kb == 0),
                                stop=(kb == KB - 1),
                            )
                    osb = pc_sb.tile([128, DIN], FP32)
                    nc.vector.tensor_tensor(
                        out=osb[:TT, :],
                        in0=ps_o[:TT, :],
                        in1=bo_b[:TT, :],
                        op=mybir.AluOpType.add,
                    )
                    nc.sync.dma_start(out=out_o[sl, :], in_=osb[:TT, :])

    nc.compile()
    return nc


_NC_CACHE = None


def _get_nc():
    global _NC_CACHE
    if _NC_CACHE is None:
        _NC_CACHE = build_bass()
    return _NC_CACHE


def kernel(x, Wi, bi, Wo, bo, codebooks):
    x = np.ascontiguousarray(np.asarray(x, dtype=np.float32))
    Wi = np.ascontiguousarray(np.asarray(Wi, dtype=np.float32))
    bi = np.ascontiguousarray(np.asarray(bi, dtype=np.float32))
    Wo = np.ascontiguousarray(np.asarray(Wo, dtype=np.float32))
    bo = np.ascontiguousarray(np.asarray(bo, dtype=np.float32))
    codebooks = np.ascontiguousarray(np.asarray(codebooks, dtype=np.float32))

    hsq = 0.5 * np.sum(
        codebooks.astype(np.float64) * codebooks.astype(np.float64), axis=-1
    ).astype(np.float32)  # [NQ, K]

    nc = _get_nc()
    shared = {
        "Wi": Wi,
        "bi": bi,
        "Wo": Wo,
        "bo": bo,
        "hsq": hsq,
    }
    for q in range(NQ):
        shared[f"cb{q}"] = np.ascontiguousarray(codebooks[q])
        shared[f"cbT{q}"] = np.ascontiguousarray(codebooks[q].T)

    in_maps = []
    for c in range(N_CORES):
        m = dict(shared)
        m["x"] = np.ascontiguousarray(
            x[c * BPC : (c + 1) * BPC].reshape(F, DIN)
        )
        in_maps.append(m)

    res = run_bass_kernel_spmd(nc, in_maps, core_ids=list(range(N_CORES)))
    results = res.results

    out = np.empty((B, T, DIN), np.float32)
    codes = np.empty((NQ, B, T), np.int32)
    sub = np.empty((NQ, B, DC, T), np.float32)
    loss_sum = 0.0
    for c in range(N_CORES):
        r = results[c]
        out[c * BPC : (c + 1) * BPC] = r["out"].reshape(BPC, T, DIN)
        codes[:, c * BPC : (c + 1) * BPC, :] = r["codes"].reshape(NQ, BPC, T)
        sub[:, c * BPC : (c + 1) * BPC] = r["sub"]
        loss_sum += float(r["lossp"].astype(np.float64).sum())

    commit_loss = np.float32(loss_sum / (NQ * B * T * DC))
    return out, codes, commit_loss, sub


if __name__ == "__main__":
    # smoke test with random inputs
    rng = np.random.default_rng(0)
    inputs = {
        "x": rng.standard_normal((B, T, DIN), dtype=np.float32),
        "Wi": rng.standard_normal((DIN, DC), dtype=np.float32) / 32.0,
        "bi": np.zeros((DC,), np.float32),
        "Wo": rng.standard_normal((DC, DIN), dtype=np.float32) / 22.6,
        "bo": np.zeros((DIN,), np.float32),
        "codebooks": rng.standard_normal((NQ, K, DC), dtype=np.float32),
    }
    outs = kernel(**inputs)
    for o in outs:
        print(np.shape(o), np.asarray(o).dtype)
